# revision 21
# baseline (speedup 1.0000x reference)
"""Block-sparse attention backward pass on 8 TRN2 NeuronCores (v11).

Sharding: head-parallel - 16 heads / 8 cores = 2 heads per core; every
core runs the same program specialized on the (replicated) block mask.

Math per (i, j) active block pair (local per-block softmax):
  S = q_i k_j^T s          U = exp(S)        l = rowsum(U)   rt = 1/l
  dA = dO_i v_j^T          rs = rowsum(U o dA)
  G = dA*rt - (rs*rt^2)    dS = U o G        (dS == Pn o (dA - rd))
  dV_j += Pn^T dO_i = U^T (dO_i o rt)        (dop = dO o rt)
  dK_j += dS^T (q_i s)     dQ_i += dS (k_j s)

Engine plan per chunk of 8 pairs (j-major pair stream):
  PE-A   : S, dA matmuls (concurrent row groups h0/h64), f32 PSUM
  ACT    : U = exp(s_ps) chunked; dAb = copy(da_ps) -> bf16 SBUF
  DVE    : 8x tensor_scalar(U)+accum -> l (4x mode); reciprocal -> rt;
           8x scalar_tensor_tensor(dAb, U)+accum -> rs (1x);
           rt2/rdr2 smalls; 8x tensor_scalar(dAb; rt, rdr2) -> G (4x)
  Pool   : dop broadcast-mult (chunked); dS = U o G (chunked TT)
  DMA    : 8x XBAR transpose dS -> dS^T (SBUF->SBUF, free engines)
  PE-B   : dV^T_j += dop_x^T U_x   (A-form: [64,128] out)
           dK_j   += dS_x^T qns_i  (B-form: stationary dS, 64-col moving)
           dQ_i   += dST_x^T kns_j (B-form: 16 live accumulators)
  PE-B for chunk c is emitted after PE-A of chunk c+LAG (software
  pipeline) so the PE never waits on the long DVE chain.

PSUM: s_ps 2 banks + da_ps 2 banks (single-buffered; freed early by
exp/dAb) + dvk pool 2 + dq accumulator 2 = 8.

Outputs: dVo [H,64,N] transposed; dKo/dQo [H,N,64] natural.
"""

import sys

sys.path.insert(0, "/opt/trn_rl_repo")

import numpy as np
import ml_dtypes

import concourse.bass as bass
import concourse.mybir as mybir
import concourse.tile as tile
from concourse import bacc
from concourse.bass_utils import run_bass_kernel_spmd
from concourse.masks import make_identity

BF16 = mybir.dt.bfloat16
F32 = mybir.dt.float32
OP = mybir.AluOpType
ACTF = mybir.ActivationFunctionType

N, D, H, DK, BLK, T = 2048, 1024, 16, 64, 128, 16
NCORES, HPC = 8, 2
SCALE = float(1.0 / np.sqrt(DK))  # tau=1
CH = 8   # pairs per chunk
LAG = 2  # software-pipeline distance between PE-A and PE-B

_BF = ml_dtypes.bfloat16


def _build(mask_key):
    mask = np.array(mask_key, dtype=np.int64).reshape(T, T)
    act_per_j = [[i for i in range(T) if mask[i, j]] for j in range(T)]
    act_per_i = [[j for j in range(T) if mask[i, j]] for i in range(T)]
    npair = int(mask.sum())
    pairs = [(i, j) for j in range(T) for i in act_per_j[j]]
    chunks = [pairs[c:c + CH] for c in range(0, npair, CH)]
    nch = len(chunks)
    # stream index of the last occurrence of each j / i
    last_of_j = {}
    last_of_i = {}
    for n, (i, j) in enumerate(pairs):
        last_of_j[j] = n
        last_of_i[i] = n
    # dq accumulator: slots 0-7 in PSUM bank A, 8-15 in bank B. Exactly one
    # start=True per (head, bank) - start resets has_written for the whole
    # bank; later slot-first matmuls overwrite via has_written=0.
    bank_ns = {0: [], 1: []}
    for n, (i, j) in enumerate(pairs):
        bank_ns[i // 8].append(n)
    dq_first = {b: ns[0] for b, ns in bank_ns.items() if ns}
    dq_last = {b: ns[-1] for b, ns in bank_ns.items() if ns}

    nc = bacc.Bacc("TRN2", target_bir_lowering=False, debug=False)

    qdo = [nc.dram_tensor(f"qdo{h}", [128, N], BF16, kind="ExternalInput")
           for h in range(HPC)]
    kv = [nc.dram_tensor(f"kv{h}", [128, N], BF16, kind="ExternalInput")
          for h in range(HPC)]
    qns = [nc.dram_tensor(f"qns{h}", [128, T * DK], BF16, kind="ExternalInput")
           for h in range(HPC)]
    kns = [nc.dram_tensor(f"kns{h}", [128, T * DK], BF16, kind="ExternalInput")
           for h in range(HPC)]
    don = [nc.dram_tensor(f"don{h}", [128, T * DK], BF16,
                          kind="ExternalInput") for h in range(HPC)]

    dVo = nc.dram_tensor("dVo", [HPC, N, DK], F32, kind="ExternalOutput")
    dKo = nc.dram_tensor("dKo", [HPC, N, DK], F32, kind="ExternalOutput")
    dQo = nc.dram_tensor("dQo", [HPC, N, DK], F32, kind="ExternalOutput")

    with tile.TileContext(nc) as tc:
        with (
            tc.tile_pool(name="const", bufs=1) as constp,
            tc.tile_pool(name="inp", bufs=1) as inp,
            tc.tile_pool(name="uwp", bufs=3) as uwp,        # [U|W] tiles
            tc.tile_pool(name="dabp", bufs=3) as dabp,      # dAb tiles
            tc.tile_pool(name="xyp", bufs=5) as xyp,        # XY tiles
            tc.tile_pool(name="dsp", bufs=5) as dsp,        # dS tiles
            tc.tile_pool(name="dstp", bufs=5) as dstp,      # dS^T tiles
            tc.tile_pool(name="dopp", bufs=4) as dopp,      # dop tiles
            tc.tile_pool(name="statp", bufs=5) as statp,
            tc.tile_pool(name="outsb", bufs=4) as outsb,
            tc.tile_pool(name="ps_s", bufs=1, space="PSUM") as ps_s,
            tc.tile_pool(name="ps_da", bufs=1, space="PSUM") as ps_da,
            tc.tile_pool(name="ps_dst", bufs=1, space="PSUM") as ps_dst,
            tc.tile_pool(name="ps_dvk", bufs=1, space="PSUM") as ps_dvk,
            tc.tile_pool(name="ps_dq", bufs=1, space="PSUM") as ps_dq,
        ):
            ident = constp.tile([128, 128], BF16)
            make_identity(nc, ident[:])
            tqdo, tkv, tqns, tkns, tdonp = [], [], [], [], []
            for h in range(HPC):
                tqdo.append(inp.tile([128, N], BF16, name=f"tqdo{h}",
                                     tag=f"qdo{h}"))
                tkv.append(inp.tile([128, N], BF16, name=f"tkv{h}",
                                    tag=f"kv{h}"))
                tqns.append(inp.tile([128, T * DK], BF16, name=f"tqns{h}",
                                     tag=f"qns{h}"))
                tkns.append(inp.tile([128, T * DK], BF16, name=f"tkns{h}",
                                     tag=f"kns{h}"))
                tdonp.append(inp.tile([128, T * DK], BF16,
                                      name=f"tdon{h}", tag=f"don{h}"))
                nc.sync.dma_start(tqdo[h][:], qdo[h][:])
                nc.sync.dma_start(tkv[h][:], kv[h][:])
                nc.sync.dma_start(tqns[h][:], qns[h][:])
                nc.sync.dma_start(tkns[h][:], kns[h][:])
                nc.sync.dma_start(tdonp[h][:], don[h][:])

            # dQ accumulator: 16 slots of [128, 64] f32 = 2 banks, shared
            # across heads (tile deps serialize the head handoff).
            dq_tile = ps_dq.tile([128, T * DK], F32, name="dqacc", tag="dq")

            # per-chunk SBUF tiles of the in-flight window
            win = {}

            dvk_st = [None, -1, 0]  # [tile, j, nacc]

            def emit_A(h, c):
                chunk = chunks[c]
                m = len(chunk)
                s_ps = ps_s.tile([128, CH * BLK], F32, tag="s")
                da_ps = ps_da.tile([128, CH * BLK], F32, tag="da")
                U = uwp.tile([128, CH * BLK], BF16, tag="U")
                dAb = dabp.tile([128, CH * BLK], BF16, tag="dAb")
                Pn = xyp.tile([128, CH * BLK], BF16, tag="Pn")
                F = dopp.tile([128, CH * BLK], BF16, tag="F")
                Hh = dabp.tile([128, CH * BLK], BF16, tag="Hh")
                dS = dsp.tile([128, CH * BLK], BF16, tag="dS")
                dST = dstp.tile([128, CH * BLK], BF16, tag="dST")
                # stb: [rt(0:CH) | rd(CH:2CH)] f32 broadcast operands
                stb = statp.tile([128, 2 * CH], F32, tag="stb")
                lf = statp.tile([128, CH], F32, tag="lf")
                rt = stb[:, 0:CH]
                rd = stb[:, CH:2 * CH]

                for x, (i, j) in enumerate(chunk):
                    cs = slice(x * BLK, (x + 1) * BLK)
                    nc.tensor.matmul(
                        s_ps[:, cs],
                        tqdo[h][0:DK, i * BLK:(i + 1) * BLK],
                        tkv[h][0:DK, j * BLK:(j + 1) * BLK],
                        start=True, stop=True, tile_position=(0, 0))
                    nc.tensor.matmul(
                        da_ps[:, cs],
                        tqdo[h][DK:128, i * BLK:(i + 1) * BLK],
                        tkv[h][DK:128, j * BLK:(j + 1) * BLK],
                        start=True, stop=True, tile_position=(DK, 0))

                nc.scalar.activation(U[:, :m * BLK], s_ps[:, :m * BLK],
                                     ACTF.Exp, scale=SCALE)
                nc.scalar.copy(dAb[:, :m * BLK], da_ps[:, :m * BLK])

                # l = grouped rowsum(U); rt = 1/l
                nc.vector.tensor_reduce(
                    lf[:, 0:m],
                    U[:, :m * BLK].rearrange("p (g x) -> p g x", x=BLK),
                    axis=mybir.AxisListType.X, op=OP.add)
                nc.vector.reciprocal_approx_fast(out=rt[:, 0:m],
                                                 in_=lf[:, 0:m])

                # Pn = U o rt (chunked broadcast on Pool)
                nc.gpsimd.tensor_tensor(
                    Pn[:, :m * BLK].rearrange("p (g x) -> p g x", x=BLK),
                    U[:, :m * BLK].rearrange("p (g x) -> p g x", x=BLK),
                    rt[:, 0:m][:, :, None].broadcast_to([128, m, BLK]),
                    op=OP.mult)
                win[(h, c)] = (Pn, dS, dST, dAb, F, Hh, stb)

            def emit_A2(h, c):
                # second elementwise stage, one pipeline slot later: the
                # DVE/Pool streams never wait mid-block this way.
                m = len(chunks[c])
                Pn, dS, dST, dAb, F, Hh, stb = win[(h, c)]
                rd = stb[:, CH:2 * CH]
                # F = Pn o dAb (2x); rd = grouped rowsum(F)
                nc.vector.tensor_tensor(F[:, :m * BLK], Pn[:, :m * BLK],
                                        dAb[:, :m * BLK], op=OP.mult)
                nc.vector.tensor_reduce(
                    rd[:, 0:m],
                    F[:, :m * BLK].rearrange("p (g x) -> p g x", x=BLK),
                    axis=mybir.AxisListType.X, op=OP.add)
                # H = Pn o rd (chunked broadcast on Pool)
                nc.gpsimd.tensor_tensor(
                    Hh[:, :m * BLK].rearrange("p (g x) -> p g x", x=BLK),
                    Pn[:, :m * BLK].rearrange("p (g x) -> p g x", x=BLK),
                    rd[:, 0:m][:, :, None].broadcast_to([128, m, BLK]),
                    op=OP.mult)
                # dS = F - H (2x)
                nc.vector.tensor_tensor(
                    dS[:, :m * BLK], F[:, :m * BLK],
                    Hh[:, :m * BLK], op=OP.subtract)

            def emit_T(h, c):
                # dS^T via PE transpose; copy to SBUF on ACT. Runs one
                # pipeline stage after emit_A so the PE does not wait on
                # the DVE chain of the same chunk.
                m = len(chunks[c])
                Pn, dS, dST = win[(h, c)][:3]
                dst_ps = ps_dst.tile([128, CH * BLK], BF16, tag="dst")
                for x in range(m):
                    cs = slice(x * BLK, (x + 1) * BLK)
                    nc.tensor.transpose(dst_ps[:, cs], dS[:, cs], ident[:])
                nc.scalar.copy(dST[:, :m * BLK], dst_ps[:, :m * BLK])

            def flush_dvk(h):
                dvk, j, _ = dvk_st
                if dvk is None:
                    return
                sb = outsb.tile([128, 2 * DK], F32, tag="dvksb")
                nc.scalar.copy(sb[:], dvk[:, 0:2 * DK])
                nc.sync.dma_start(dKo[h, j * BLK:(j + 1) * BLK, :],
                                  sb[:, 0:DK])
                nc.sync.dma_start(dVo[h, j * BLK:(j + 1) * BLK, :],
                                  sb[:, DK:2 * DK])
                dvk_st[0] = None

            def emit_B(h, c):
                chunk = chunks[c]
                Pn, dS, dST = win.pop((h, c))[:3]
                for x, (i, j) in enumerate(chunk):
                    n = c * CH + x
                    cs = slice(x * BLK, (x + 1) * BLK)
                    if j != dvk_st[1] or dvk_st[0] is None:
                        flush_dvk(h)
                        # bank-sized tile: each buf must own a full PSUM
                        # bank (start=True resets has_written bank-wide)
                        dvk_st[0] = ps_dvk.tile([128, 512], F32,
                                                name="dvkps", tag="dvk")
                        dvk_st[1] = j
                        dvk_st[2] = 0
                    dvk = dvk_st[0]
                    npair_j = len(act_per_j[j])
                    first = dvk_st[2] == 0
                    last = dvk_st[2] == npair_j - 1
                    # dK_j += dS_x^T qns_i  ([128,64] out). Only this first
                    # matmul carries start=True: it spans all 128
                    # partitions, so the bank-wide has_written clear covers
                    # the dV region too; dV's first write then lands on
                    # has_written=0 (overwrite).
                    nc.tensor.matmul(
                        dvk[:, 0:DK],
                        dS[:, cs],
                        tqns[h][:, i * DK:(i + 1) * DK],
                        start=first, stop=last, skip_group_check=True)
                    # dV_j += Pn_x^T dOnat_i  ([128,64] out)
                    nc.tensor.matmul(
                        dvk[:, DK:2 * DK],
                        Pn[:, cs],
                        tdonp[h][:, i * DK:(i + 1) * DK],
                        start=False, stop=last, skip_group_check=True)
                    dvk_st[2] += 1
                    # dQ_i += dST_x^T kns_j  (slot i of dq_tile)
                    nc.tensor.matmul(
                        dq_tile[:, i * DK:(i + 1) * DK],
                        dST[:, cs],
                        tkns[h][:, j * DK:(j + 1) * DK],
                        start=(dq_first[i // 8] == n),
                        stop=(dq_last[i // 8] == n),
                        skip_group_check=True)
                    if n == last_of_j[j]:
                        flush_dvk(h)

            def flush_dq(h):
                sbq = outsb.tile([128, T * DK], F32, tag="dqsb")
                nc.scalar.copy(sbq[:], dq_tile[:])
                nc.sync.dma_start(
                    dQo[h].rearrange("(t p) d -> p t d", p=BLK),
                    sbq[:].rearrange("p (t d) -> p t d", d=DK))

            # flat (h, c) stream; 4-stage software pipeline:
            # A2(idx-1) | A1(idx) | T(idx-2) | B(idx-3), crossing head
            # boundaries so no engine drains between heads.
            seq = [(h, c) for h in range(HPC) for c in range(nch)]

            def tail(idx):
                if 0 <= idx - 1 < len(seq):
                    emit_A2(*seq[idx - 1])
                if 0 <= idx - 2 < len(seq):
                    emit_T(*seq[idx - 2])
                if 0 <= idx - 3 < len(seq):
                    hb, cb = seq[idx - 3]
                    emit_B(hb, cb)
                    if cb == nch - 1:
                        flush_dq(hb)

            for idx, (h, c) in enumerate(seq):
                tail(idx)
                emit_A(h, c)
            for idx in range(len(seq), len(seq) + 4):
                tail(idx)

    nc.compile()
    return nc, npair, pairs


_prog_cache = {}


def _get_prog(mask):
    key = tuple(int(x) for x in np.asarray(mask).astype(np.int64).ravel())
    if key not in _prog_cache:
        _prog_cache[key] = _build(key)
    return _prog_cache[key]


def kernel(q, k, v, dO, block_sparse_mask, _trace=False):
    q = np.ascontiguousarray(np.asarray(q, dtype=np.float32))
    k = np.ascontiguousarray(np.asarray(k, dtype=np.float32))
    v = np.ascontiguousarray(np.asarray(v, dtype=np.float32))
    dO = np.ascontiguousarray(np.asarray(dO, dtype=np.float32))
    mask = np.asarray(block_sparse_mask)

    nc, npair, pairs = _get_prog(mask)

    def tlay(x, g):  # head g of (1,N,D) -> [64, N] transposed bf16
        return np.ascontiguousarray(
            x[0, :, g * DK:(g + 1) * DK].T).astype(_BF)

    def nlay(x, g, scale):  # head g natural -> [128, T*DK]
        y = (x[0, :, g * DK:(g + 1) * DK] * scale).reshape(T, BLK, DK)
        return np.ascontiguousarray(
            y.transpose(1, 0, 2).reshape(BLK, T * DK)).astype(_BF)

    in_maps = []
    for c in range(NCORES):
        im = {}
        for h in range(HPC):
            g = c * HPC + h
            im[f"qdo{h}"] = np.ascontiguousarray(
                np.concatenate([tlay(q, g), tlay(dO, g)], axis=0))
            im[f"kv{h}"] = np.ascontiguousarray(
                np.concatenate([tlay(k, g), tlay(v, g)], axis=0))
            im[f"qns{h}"] = nlay(q, g, SCALE)
            im[f"kns{h}"] = nlay(k, g, SCALE)
            im[f"don{h}"] = nlay(dO, g, 1.0)
        in_maps.append(im)

    res = run_bass_kernel_spmd(nc, in_maps, list(range(NCORES)), trace=_trace)
    if _trace:
        kernel.last_exec_time_ns = res.exec_time_ns
        kernel.last_res = res

    m64 = np.asarray(mask).astype(np.int64)
    empty_i = [i for i in range(T) if not m64[i, :].any()]
    empty_j = [j for j in range(T) if not m64[:, j].any()]

    dQ = np.empty((1, N, D), np.float32)
    dK = np.empty((1, N, D), np.float32)
    dV = np.empty((1, N, D), np.float32)
    for c in range(NCORES):
        r = res.results[c]
        for h in range(HPC):
            g = c * HPC + h
            dQ[0, :, g * DK:(g + 1) * DK] = r["dQo"][h]
            dK[0, :, g * DK:(g + 1) * DK] = r["dKo"][h]
            dV[0, :, g * DK:(g + 1) * DK] = r["dVo"][h]
    for i in empty_i:
        dQ[0, i * BLK:(i + 1) * BLK, :] = 0.0
    for j in empty_j:
        dK[0, j * BLK:(j + 1) * BLK, :] = 0.0
        dV[0, j * BLK:(j + 1) * BLK, :] = 0.0
    return dQ, dK, dV


# revision 23
# speedup vs baseline: 1.0993x; 1.0993x over previous
"""Block-sparse attention backward pass on 8 TRN2 NeuronCores (v11).

Sharding: head-parallel - 16 heads / 8 cores = 2 heads per core; every
core runs the same program specialized on the (replicated) block mask.

Math per (i, j) active block pair (local per-block softmax):
  S = q_i k_j^T s          U = exp(S)        l = rowsum(U)   rt = 1/l
  dA = dO_i v_j^T          rs = rowsum(U o dA)
  G = dA*rt - (rs*rt^2)    dS = U o G        (dS == Pn o (dA - rd))
  dV_j += Pn^T dO_i = U^T (dO_i o rt)        (dop = dO o rt)
  dK_j += dS^T (q_i s)     dQ_i += dS (k_j s)

Engine plan per chunk of 8 pairs (j-major pair stream):
  PE-A   : S, dA matmuls (concurrent row groups h0/h64), f32 PSUM
  ACT    : U = exp(s_ps) chunked; dAb = copy(da_ps) -> bf16 SBUF
  DVE    : 8x tensor_scalar(U)+accum -> l (4x mode); reciprocal -> rt;
           8x scalar_tensor_tensor(dAb, U)+accum -> rs (1x);
           rt2/rdr2 smalls; 8x tensor_scalar(dAb; rt, rdr2) -> G (4x)
  Pool   : dop broadcast-mult (chunked); dS = U o G (chunked TT)
  DMA    : 8x XBAR transpose dS -> dS^T (SBUF->SBUF, free engines)
  PE-B   : dV^T_j += dop_x^T U_x   (A-form: [64,128] out)
           dK_j   += dS_x^T qns_i  (B-form: stationary dS, 64-col moving)
           dQ_i   += dST_x^T kns_j (B-form: 16 live accumulators)
  PE-B for chunk c is emitted after PE-A of chunk c+LAG (software
  pipeline) so the PE never waits on the long DVE chain.

PSUM: s_ps 2 banks + da_ps 2 banks (single-buffered; freed early by
exp/dAb) + dvk pool 2 + dq accumulator 2 = 8.

Outputs: dVo [H,64,N] transposed; dKo/dQo [H,N,64] natural.
"""

import sys

sys.path.insert(0, "/opt/trn_rl_repo")

import numpy as np
import ml_dtypes

import concourse.bass as bass
import concourse.mybir as mybir
import concourse.tile as tile
from concourse import bacc
from concourse.bass_utils import run_bass_kernel_spmd
from concourse.masks import make_identity

BF16 = mybir.dt.bfloat16
F32 = mybir.dt.float32
OP = mybir.AluOpType
ACTF = mybir.ActivationFunctionType

N, D, H, DK, BLK, T = 2048, 1024, 16, 64, 128, 16
NCORES, HPC = 8, 2
SCALE = float(1.0 / np.sqrt(DK))  # tau=1
CH = 8   # pairs per chunk
LAG = 2  # software-pipeline distance between PE-A and PE-B

_BF = ml_dtypes.bfloat16


def _build(mask_key):
    mask = np.array(mask_key, dtype=np.int64).reshape(T, T)
    act_per_j = [[i for i in range(T) if mask[i, j]] for j in range(T)]
    act_per_i = [[j for j in range(T) if mask[i, j]] for i in range(T)]
    npair = int(mask.sum())
    pairs = [(i, j) for j in range(T) for i in act_per_j[j]]
    chunks = [pairs[c:c + CH] for c in range(0, npair, CH)]
    nch = len(chunks)
    # stream index of the last occurrence of each j / i
    last_of_j = {}
    last_of_i = {}
    for n, (i, j) in enumerate(pairs):
        last_of_j[j] = n
        last_of_i[i] = n
    # dq accumulator: slots 0-7 in PSUM bank A, 8-15 in bank B. Exactly one
    # start=True per (head, bank) - start resets has_written for the whole
    # bank; later slot-first matmuls overwrite via has_written=0.
    bank_ns = {0: [], 1: []}
    for n, (i, j) in enumerate(pairs):
        bank_ns[i // 8].append(n)
    dq_first = {b: ns[0] for b, ns in bank_ns.items() if ns}
    dq_last = {b: ns[-1] for b, ns in bank_ns.items() if ns}

    nc = bacc.Bacc("TRN2", target_bir_lowering=False, debug=False)

    qdo = [nc.dram_tensor(f"qdo{h}", [128, N], BF16, kind="ExternalInput")
           for h in range(HPC)]
    kv = [nc.dram_tensor(f"kv{h}", [128, N], BF16, kind="ExternalInput")
          for h in range(HPC)]
    qns = [nc.dram_tensor(f"qns{h}", [128, T * DK], BF16, kind="ExternalInput")
           for h in range(HPC)]
    kns = [nc.dram_tensor(f"kns{h}", [128, T * DK], BF16, kind="ExternalInput")
           for h in range(HPC)]
    don = [nc.dram_tensor(f"don{h}", [128, T * DK], BF16,
                          kind="ExternalInput") for h in range(HPC)]

    dVo = nc.dram_tensor("dVo", [HPC, N, DK], F32, kind="ExternalOutput")
    dKo = nc.dram_tensor("dKo", [HPC, N, DK], F32, kind="ExternalOutput")
    dQo = nc.dram_tensor("dQo", [HPC, N, DK], F32, kind="ExternalOutput")

    with tile.TileContext(nc) as tc:
        with (
            tc.tile_pool(name="const", bufs=1) as constp,
            tc.tile_pool(name="inp", bufs=1) as inp,
            tc.tile_pool(name="uwp", bufs=3) as uwp,        # [U|W] tiles
            tc.tile_pool(name="dabp", bufs=4) as dabp,      # dAb tiles
            tc.tile_pool(name="xyp", bufs=8) as xyp,        # XY tiles
            tc.tile_pool(name="dsp", bufs=4) as dsp,        # dS tiles
            tc.tile_pool(name="dstp", bufs=3) as dstp,      # dS^T tiles
            tc.tile_pool(name="dopp", bufs=4) as dopp,      # dop tiles
            tc.tile_pool(name="statp", bufs=6) as statp,
            tc.tile_pool(name="outsb", bufs=4) as outsb,
            tc.tile_pool(name="ps_s", bufs=1, space="PSUM") as ps_s,
            tc.tile_pool(name="ps_da", bufs=1, space="PSUM") as ps_da,
            tc.tile_pool(name="ps_dst", bufs=1, space="PSUM") as ps_dst,
            tc.tile_pool(name="ps_dvk", bufs=1, space="PSUM") as ps_dvk,
            tc.tile_pool(name="ps_dq", bufs=1, space="PSUM") as ps_dq,
        ):
            ident = constp.tile([128, 128], BF16)
            make_identity(nc, ident[:])
            tqdo, tkv, tqns, tkns, tdonp = [], [], [], [], []
            for h in range(HPC):
                tqdo.append(inp.tile([128, N], BF16, name=f"tqdo{h}",
                                     tag=f"qdo{h}"))
                tkv.append(inp.tile([128, N], BF16, name=f"tkv{h}",
                                    tag=f"kv{h}"))
                tqns.append(inp.tile([128, T * DK], BF16, name=f"tqns{h}",
                                     tag=f"qns{h}"))
                tkns.append(inp.tile([128, T * DK], BF16, name=f"tkns{h}",
                                     tag=f"kns{h}"))
                tdonp.append(inp.tile([128, T * DK], BF16,
                                      name=f"tdon{h}", tag=f"don{h}"))
                nc.sync.dma_start(tqdo[h][:], qdo[h][:])
                nc.sync.dma_start(tkv[h][:], kv[h][:])
                nc.sync.dma_start(tqns[h][:], qns[h][:])
                nc.sync.dma_start(tkns[h][:], kns[h][:])
                nc.sync.dma_start(tdonp[h][:], don[h][:])

            # dQ accumulator: 16 slots of [128, 64] f32 = 2 banks, shared
            # across heads (tile deps serialize the head handoff).
            dq_tile = ps_dq.tile([128, T * DK], F32, name="dqacc", tag="dq")

            # per-chunk SBUF tiles of the in-flight window
            win = {}

            dvk_st = [None, -1, 0]  # [tile, j, nacc]

            def emit_A(h, c):
                chunk = chunks[c]
                m = len(chunk)
                s_ps = ps_s.tile([128, CH * BLK], F32, tag="s")
                da_ps = ps_da.tile([128, CH * BLK], F32, tag="da")
                U = uwp.tile([128, CH * BLK], BF16, tag="U")
                dAb = dabp.tile([128, CH * BLK], BF16, tag="dAb")
                Pn = xyp.tile([128, CH * BLK], BF16, tag="Pn")
                F = dopp.tile([128, CH * BLK], BF16, tag="F")
                Hh = dabp.tile([128, CH * BLK], BF16, tag="Hh")
                dS = dsp.tile([128, CH * BLK], BF16, tag="dS")
                dST = dstp.tile([128, CH * BLK], BF16, tag="dST")
                # stb: [rt(0:CH) | rd(CH:2CH)] f32 broadcast operands
                stb = statp.tile([128, 2 * CH], F32, tag="stb")
                lf = statp.tile([128, CH], F32, tag="lf")
                rt = stb[:, 0:CH]
                rd = stb[:, CH:2 * CH]

                for x, (i, j) in enumerate(chunk):
                    cs = slice(x * BLK, (x + 1) * BLK)
                    nc.tensor.matmul(
                        s_ps[:, cs],
                        tqdo[h][0:DK, i * BLK:(i + 1) * BLK],
                        tkv[h][0:DK, j * BLK:(j + 1) * BLK],
                        start=True, stop=True, tile_position=(0, 0))
                    nc.tensor.matmul(
                        da_ps[:, cs],
                        tqdo[h][DK:128, i * BLK:(i + 1) * BLK],
                        tkv[h][DK:128, j * BLK:(j + 1) * BLK],
                        start=True, stop=True, tile_position=(DK, 0))

                nc.scalar.activation(U[:, :m * BLK], s_ps[:, :m * BLK],
                                     ACTF.Exp, scale=SCALE)
                nc.scalar.copy(dAb[:, :m * BLK], da_ps[:, :m * BLK])

                # l = grouped rowsum(U); rt = 1/l
                nc.vector.tensor_reduce(
                    lf[:, 0:m],
                    U[:, :m * BLK].rearrange("p (g x) -> p g x", x=BLK),
                    axis=mybir.AxisListType.X, op=OP.add)
                nc.vector.reciprocal_approx_fast(out=rt[:, 0:m],
                                                 in_=lf[:, 0:m])

                # Pn = U o rt (chunked broadcast on Pool)
                nc.gpsimd.tensor_tensor(
                    Pn[:, :m * BLK].rearrange("p (g x) -> p g x", x=BLK),
                    U[:, :m * BLK].rearrange("p (g x) -> p g x", x=BLK),
                    rt[:, 0:m][:, :, None].broadcast_to([128, m, BLK]),
                    op=OP.mult)
                win[(h, c)] = (Pn, dS, dST, dAb, F, Hh, stb)

            def emit_A2(h, c):
                # +2 stages after A1: Pn is 2 blocks old, so the DVE
                # stream never waits on the Pool here.
                m = len(chunks[c])
                Pn, dS, dST, dAb, F, Hh, stb = win[(h, c)]
                rd = stb[:, CH:2 * CH]
                # F = Pn o dAb (2x); rd = grouped rowsum(F)
                nc.vector.tensor_tensor(F[:, :m * BLK], Pn[:, :m * BLK],
                                        dAb[:, :m * BLK], op=OP.mult)
                nc.vector.tensor_reduce(
                    rd[:, 0:m],
                    F[:, :m * BLK].rearrange("p (g x) -> p g x", x=BLK),
                    axis=mybir.AxisListType.X, op=OP.add)
                # H = Pn o rd (chunked broadcast on Pool)
                nc.gpsimd.tensor_tensor(
                    Hh[:, :m * BLK].rearrange("p (g x) -> p g x", x=BLK),
                    Pn[:, :m * BLK].rearrange("p (g x) -> p g x", x=BLK),
                    rd[:, 0:m][:, :, None].broadcast_to([128, m, BLK]),
                    op=OP.mult)

            def emit_A3(h, c):
                # +4 stages: H is 2 blocks old.
                m = len(chunks[c])
                Pn, dS, dST, dAb, F, Hh, stb = win[(h, c)]
                # dS = F - H (2x)
                nc.vector.tensor_tensor(
                    dS[:, :m * BLK], F[:, :m * BLK],
                    Hh[:, :m * BLK], op=OP.subtract)

            def emit_T(h, c):
                # dS^T via PE transpose; copy to SBUF on ACT. Runs one
                # pipeline stage after emit_A so the PE does not wait on
                # the DVE chain of the same chunk.
                m = len(chunks[c])
                Pn, dS, dST = win[(h, c)][:3]
                dst_ps = ps_dst.tile([128, CH * BLK], BF16, tag="dst")
                for x in range(m):
                    cs = slice(x * BLK, (x + 1) * BLK)
                    nc.tensor.transpose(dst_ps[:, cs], dS[:, cs], ident[:])
                nc.scalar.copy(dST[:, :m * BLK], dst_ps[:, :m * BLK])

            def flush_dvk(h):
                dvk, j, _ = dvk_st
                if dvk is None:
                    return
                sb = outsb.tile([128, 2 * DK], F32, tag="dvksb")
                nc.scalar.copy(sb[:], dvk[:, 0:2 * DK])
                nc.sync.dma_start(dKo[h, j * BLK:(j + 1) * BLK, :],
                                  sb[:, 0:DK])
                nc.sync.dma_start(dVo[h, j * BLK:(j + 1) * BLK, :],
                                  sb[:, DK:2 * DK])
                dvk_st[0] = None

            def emit_B(h, c):
                chunk = chunks[c]
                Pn, dS, dST = win.pop((h, c))[:3]
                for x, (i, j) in enumerate(chunk):
                    n = c * CH + x
                    cs = slice(x * BLK, (x + 1) * BLK)
                    if j != dvk_st[1] or dvk_st[0] is None:
                        flush_dvk(h)
                        # bank-sized tile: each buf must own a full PSUM
                        # bank (start=True resets has_written bank-wide)
                        dvk_st[0] = ps_dvk.tile([128, 512], F32,
                                                name="dvkps", tag="dvk")
                        dvk_st[1] = j
                        dvk_st[2] = 0
                    dvk = dvk_st[0]
                    npair_j = len(act_per_j[j])
                    first = dvk_st[2] == 0
                    last = dvk_st[2] == npair_j - 1
                    # dK_j += dS_x^T qns_i  ([128,64] out). Only this first
                    # matmul carries start=True: it spans all 128
                    # partitions, so the bank-wide has_written clear covers
                    # the dV region too; dV's first write then lands on
                    # has_written=0 (overwrite).
                    nc.tensor.matmul(
                        dvk[:, 0:DK],
                        dS[:, cs],
                        tqns[h][:, i * DK:(i + 1) * DK],
                        start=first, stop=last, skip_group_check=True)
                    # dV_j += Pn_x^T dOnat_i  ([128,64] out)
                    nc.tensor.matmul(
                        dvk[:, DK:2 * DK],
                        Pn[:, cs],
                        tdonp[h][:, i * DK:(i + 1) * DK],
                        start=False, stop=last, skip_group_check=True)
                    dvk_st[2] += 1
                    # dQ_i += dST_x^T kns_j  (slot i of dq_tile)
                    nc.tensor.matmul(
                        dq_tile[:, i * DK:(i + 1) * DK],
                        dST[:, cs],
                        tkns[h][:, j * DK:(j + 1) * DK],
                        start=(dq_first[i // 8] == n),
                        stop=(dq_last[i // 8] == n),
                        skip_group_check=True)
                    if n == last_of_j[j]:
                        flush_dvk(h)

            def flush_dq(h):
                sbq = outsb.tile([128, T * DK], F32, tag="dqsb")
                nc.scalar.copy(sbq[:], dq_tile[:])
                nc.sync.dma_start(
                    dQo[h].rearrange("(t p) d -> p t d", p=BLK),
                    sbq[:].rearrange("p (t d) -> p t d", d=DK))

            # flat (h, c) stream; 6-stage software pipeline so every
            # cross-engine dependency is at least two blocks old:
            # A1(idx) | A2(idx-2) | A3(idx-4) | T(idx-5) | B(idx-6)
            seq = [(h, c) for h in range(HPC) for c in range(nch)]

            def stage(fn, idx):
                if 0 <= idx < len(seq):
                    fn(*seq[idx])

            for idx in range(len(seq) + 7):
                stage(emit_A2, idx - 2)
                if idx < len(seq):
                    emit_A(*seq[idx])
                stage(emit_A3, idx - 4)
                stage(emit_T, idx - 5)
                if 0 <= idx - 6 < len(seq):
                    hb, cb = seq[idx - 6]
                    emit_B(hb, cb)
                    if cb == nch - 1:
                        flush_dq(hb)

    nc.compile()
    return nc, npair, pairs


_prog_cache = {}


def _get_prog(mask):
    key = tuple(int(x) for x in np.asarray(mask).astype(np.int64).ravel())
    if key not in _prog_cache:
        _prog_cache[key] = _build(key)
    return _prog_cache[key]


def kernel(q, k, v, dO, block_sparse_mask, _trace=False):
    q = np.ascontiguousarray(np.asarray(q, dtype=np.float32))
    k = np.ascontiguousarray(np.asarray(k, dtype=np.float32))
    v = np.ascontiguousarray(np.asarray(v, dtype=np.float32))
    dO = np.ascontiguousarray(np.asarray(dO, dtype=np.float32))
    mask = np.asarray(block_sparse_mask)

    nc, npair, pairs = _get_prog(mask)

    def tlay(x, g):  # head g of (1,N,D) -> [64, N] transposed bf16
        return np.ascontiguousarray(
            x[0, :, g * DK:(g + 1) * DK].T).astype(_BF)

    def nlay(x, g, scale):  # head g natural -> [128, T*DK]
        y = (x[0, :, g * DK:(g + 1) * DK] * scale).reshape(T, BLK, DK)
        return np.ascontiguousarray(
            y.transpose(1, 0, 2).reshape(BLK, T * DK)).astype(_BF)

    in_maps = []
    for c in range(NCORES):
        im = {}
        for h in range(HPC):
            g = c * HPC + h
            im[f"qdo{h}"] = np.ascontiguousarray(
                np.concatenate([tlay(q, g), tlay(dO, g)], axis=0))
            im[f"kv{h}"] = np.ascontiguousarray(
                np.concatenate([tlay(k, g), tlay(v, g)], axis=0))
            im[f"qns{h}"] = nlay(q, g, SCALE)
            im[f"kns{h}"] = nlay(k, g, SCALE)
            im[f"don{h}"] = nlay(dO, g, 1.0)
        in_maps.append(im)

    res = run_bass_kernel_spmd(nc, in_maps, list(range(NCORES)), trace=_trace)
    if _trace:
        kernel.last_exec_time_ns = res.exec_time_ns
        kernel.last_res = res

    m64 = np.asarray(mask).astype(np.int64)
    empty_i = [i for i in range(T) if not m64[i, :].any()]
    empty_j = [j for j in range(T) if not m64[:, j].any()]

    dQ = np.empty((1, N, D), np.float32)
    dK = np.empty((1, N, D), np.float32)
    dV = np.empty((1, N, D), np.float32)
    for c in range(NCORES):
        r = res.results[c]
        for h in range(HPC):
            g = c * HPC + h
            dQ[0, :, g * DK:(g + 1) * DK] = r["dQo"][h]
            dK[0, :, g * DK:(g + 1) * DK] = r["dKo"][h]
            dV[0, :, g * DK:(g + 1) * DK] = r["dVo"][h]
    for i in empty_i:
        dQ[0, i * BLK:(i + 1) * BLK, :] = 0.0
    for j in empty_j:
        dK[0, j * BLK:(j + 1) * BLK, :] = 0.0
        dV[0, j * BLK:(j + 1) * BLK, :] = 0.0
    return dQ, dK, dV
